# revision 1
# baseline (speedup 1.0000x reference)
"""Trainium2 Bass kernel for nn_AttentionHead (B=4, S=4096, D=512).

reference:
    K = x @ Wk.T; Q = x @ Wq.T; V = x @ Wv.T            # [B,S,D]
    scores[b,s,t] = <K[b,s], Q[b,t]> / sqrt(D)
    scores[b,:,t] = -1e12 where mask[b,t]==0
    out = softmax(scores, axis=t) @ V                    # [B,S,D]

Sharding: 8 cores = 4 batches x 2 sequence halves (rows s of the score
matrix). No collectives; each core computes Q^T/V for the full sequence of
its batch and K^T for its s-half only. (A pairwise AllGather variant that
deduplicates the Q/V projections was measured 230us SLOWER -- 2-core
collective_compute runs at ~40GB/s effective -- so the small duplicated
projection work is the right trade.)

Device dataflow (per core), all matmuls in float32r (full PE rate at
N=512, ~1.6e-4 rounding per matmul; plain fp32 matmul is 4x slower):
    phase 1: K^T[d,s] = WkT-tile.T @ x^T (s-half)   -- done first so
             phase 2 can start earliest
             Q^T[d,t], V[t,d] from the full-sequence x^T
    phase 2: per s-chunk of 512, for each t-tile of 128:
             S^T[t,s]  = sum_d Q^T-tile.T @ K^T            (PSUM, 4 MMs)
             P^T       = exp(S^T/sqrt(D) + mbias[t])       (ACT -> f32r)
             out^T[d,s]+= V-tile.T @ P^T                   (4 MMs, PSUM acc)
             den128    += P^T                              (DVE, off the PE)
             epilogue: den = ones.T @ den128 (1 MM), recip, broadcast via a
             rank-1 matmul, out^T *= 1/den, DMA out^T.

Masking: mbias[t] = (mask[t]-1)*1e9 is added inside the EXP, so masked keys
underflow to exactly 0 -- identical to the reference's -1e12 fill followed
by softmax (requires >=1 unmasked key per batch, which random 0/1 masks
over 4096 positions guarantee), and it makes the plain column-sum of P^T
the correct denominator with no extra matmuls against a mask column.

Host passes x^T / W^T layouts (pure permutations; all FLOPs stay on
device). The f32r DRAM declaration lets raw fp32 bits feed f32r matmuls
directly (verified bit-path: end-to-end err ~6e-4).
"""

import numpy as np

import concourse.bacc as bacc
import concourse.mybir as mybir
from concourse.bass_utils import run_bass_kernel_spmd
from concourse.tile import TileContext

B, S, D = 4, 4096, 512
SH = S // 2          # per-core s rows (half sequence)
P = 128              # partition tile
CH = 512             # free-dim chunk
KD = D // P          # 4 contraction tiles over d
NT = S // P          # 32 t-tiles
SCALE = 1.0 / float(np.sqrt(D))

F32 = mybir.dt.float32
F32R = mybir.dt.float32r
COPY = mybir.ActivationFunctionType.Copy
EXP = mybir.ActivationFunctionType.Exp

VW = D               # V tile width (mask folded into EXP bias instead)

_CACHE = {}


RG = [[0, 1], [2, 3], [4, 5], [6, 7]]   # core pairs sharing one batch


def _build():
    nc = bacc.Bacc(num_devices=8)
    xT = nc.declare_dram_parameter("xT", [D, S], F32R, isOutput=False)
    xsT = nc.declare_dram_parameter("xsT", [D, SH], F32R, isOutput=False)
    wqT = nc.declare_dram_parameter("wqT", [D, D], F32R, isOutput=False)
    wkT = nc.declare_dram_parameter("wkT", [D, D], F32R, isOutput=False)
    wvT = nc.declare_dram_parameter("wvT", [D, D], F32R, isOutput=False)
    maskT = nc.declare_dram_parameter("maskT", [P, NT], F32, isOutput=False)
    outT = nc.declare_dram_parameter("outT", [D, SH], F32, isOutput=True)

    with TileContext(nc) as tc:
        with tc.tile_pool(name="pers", bufs=1) as pers:
            qT = pers.tile([P, KD * S], F32R)        # d-tile j at [:, j*S:]
            kT = pers.tile([P, KD * SH], F32R)
            vA = pers.tile([P, NT * VW], F32R)       # t-tile i at [:, i*VW:]
            mk = pers.tile([P, NT], F32)
            ones = pers.tile([1, P], F32R)
            ones32 = pers.tile([1, P], F32)
            onec = pers.tile([P, 1], F32R)
            onec32 = pers.tile([P, 1], F32)
            mbias = pers.tile([P, NT], F32)

            # ---------------- phase 1: projections ----------------
            with tc.tile_pool(name="stage", bufs=1) as stage, \
                 tc.tile_pool(name="ppsum", bufs=3, space="PSUM") as ppsum:
                wq = stage.tile([P, KD * D], F32R, tag="wq")
                wk = stage.tile([P, KD * D], F32R, tag="wk")
                wv = stage.tile([P, KD * D], F32R, tag="wv")
                # wk + first xsT chunk first, interleaved across the three
                # DMA-capable queues in k-tile order, so the first K matmul's
                # dependencies (wk tile 0 + x tile 0) land in parallel and
                # the later k-tiles arrive in consumption order
                # PE warm-up: dummy matmuls into a trash PSUM bank while the
                # first DMAs are in flight -- keeps the HAM clock-gate at
                # 2.4GHz so the real matmuls start warm instead of paying
                # the ~3.4us half-rate ramp
                warm32 = stage.tile([P, CH], F32, tag="warm32")
                warm = stage.tile([P, CH], F32R, tag="warm")
                nc.vector.memset(warm32, 0.0)
                nc.vector.tensor_copy(out=warm, in_=warm32)
                for r in range(24):
                    wps = ppsum.tile([P, CH], F32, tag="warm", bufs=2,
                                     name="wps")
                    nc.tensor.matmul(wps, warm[:, 0:P], warm,
                                     start=True, stop=True)

                engs = [nc.sync, nc.gpsimd, nc.scalar]
                xr0 = stage.tile([P, KD * CH], F32R, tag="xr", bufs=2,
                                 name="xr0")
                # pair each k-tile's (wk, x) DMAs on ONE queue: the first K
                # matmul of k-tile j then needs a single queue-sem wait and
                # can start as soon as ITS pair lands, not after all eight
                for j in range(KD):
                    eng = engs[j % 3]
                    eng.dma_start(
                        out=wk[:, j * D:(j + 1) * D],
                        in_=wkT[j * P:(j + 1) * P, :])
                    eng.dma_start(
                        out=xr0[:, j * CH:(j + 1) * CH],
                        in_=xsT[j * P:(j + 1) * P, 0:CH])
                for j in range(KD):
                    nc.sync.dma_start(out=wq[:, j * D:(j + 1) * D],
                                      in_=wqT[j * P:(j + 1) * P, :])
                    nc.gpsimd.dma_start(out=wv[:, j * D:(j + 1) * D],
                                        in_=wvT[j * P:(j + 1) * P, :])

                # constants + mask bias (off the first-wave critical path)
                nc.scalar.dma_start(out=mk, in_=maskT[:, :])
                nc.vector.memset(ones32, 1.0)
                nc.vector.tensor_copy(out=ones, in_=ones32)
                nc.vector.memset(onec32, 1.0)
                nc.vector.tensor_copy(out=onec, in_=onec32)
                # mbias[p, i] = (mask-1)*1e9: 0 where kept, -1e9 where
                # masked; exp(score*scale + mbias) underflows to exactly 0
                nc.vector.tensor_scalar(mbias, mk, -1.0, 1.0e9,
                                        mybir.AluOpType.add,
                                        mybir.AluOpType.mult)

                # K^T first (phase 2's first score groups need it earliest)
                for c in range(SH // CH):
                    if c == 0:
                        xr = xr0
                    else:
                        xr = stage.tile([P, KD * CH], F32R, tag="xr", bufs=2,
                                        name="xrk")
                        for j in range(KD):
                            nc.sync.dma_start(
                                out=xr[:, j * CH:(j + 1) * CH],
                                in_=xsT[j * P:(j + 1) * P, c * CH:(c + 1) * CH])
                    for jo in range(KD):
                        pq = ppsum.tile([P, CH], F32, tag="pq", name="pqk")
                        for kd in range(KD):
                            nc.tensor.matmul(
                                pq,
                                wk[:, kd * D + jo * P: kd * D + (jo + 1) * P],
                                xr[:, kd * CH:(kd + 1) * CH],
                                start=(kd == 0), stop=(kd == KD - 1))
                        nc.scalar.activation(
                            out=kT[:, jo * SH + c * CH: jo * SH + (c + 1) * CH],
                            in_=pq, func=COPY)

                # Q^T and V from full x^T, chunk by chunk
                for c in range(S // CH):
                    xr = stage.tile([P, KD * CH], F32R, tag="xr", bufs=2,
                                    name="xrq")
                    for j in range(KD):
                        nc.sync.dma_start(
                            out=xr[:, j * CH:(j + 1) * CH],
                            in_=xT[j * P:(j + 1) * P, c * CH:(c + 1) * CH])
                    for jo in range(KD):
                        pq = ppsum.tile([P, CH], F32, tag="pq")
                        for kd in range(KD):
                            nc.tensor.matmul(
                                pq,
                                wq[:, kd * D + jo * P: kd * D + (jo + 1) * P],
                                xr[:, kd * CH:(kd + 1) * CH],
                                start=(kd == 0), stop=(kd == KD - 1))
                        nc.scalar.activation(
                            out=qT[:, jo * S + c * CH: jo * S + (c + 1) * CH],
                            in_=pq, func=COPY)
                    for tt in range(CH // P):
                        ti = c * (CH // P) + tt
                        pv = ppsum.tile([P, D], F32, tag="pv")
                        for kd in range(KD):
                            nc.tensor.matmul(
                                pv,
                                xr[:, kd * CH + tt * P: kd * CH + (tt + 1) * P],
                                wv[:, kd * D:(kd + 1) * D],
                                start=(kd == 0), stop=(kd == KD - 1))
                        nc.scalar.activation(
                            out=vA[:, ti * VW: ti * VW + D], in_=pv,
                            func=COPY)

            # ---------------- phase 2: attention ----------------
            with tc.tile_pool(name="att", bufs=1) as att, \
                 tc.tile_pool(name="apsum", bufs=1, space="PSUM") as apsum:

                for sc in range(SH // CH):
                    opsum = [apsum.tile([P, CH], F32, tag=f"o{d}",
                                        name=f"opsum{d}")
                             for d in range(KD)]
                    # mask weights: P^T sum accumulated on DVE (not PE)
                    den128 = att.tile([P, CH], F32R, tag="den128")

                    def s_group(ti, sc=sc):
                        ss = apsum.tile([P, CH], F32, tag="s", bufs=3)
                        for kd in range(KD):
                            nc.tensor.matmul(
                                ss,
                                qT[:, kd * S + ti * P: kd * S + (ti + 1) * P],
                                kT[:, kd * SH + sc * CH: kd * SH + (sc + 1) * CH],
                                start=(kd == 0), stop=(kd == KD - 1))
                        return ss

                    ss_cur = s_group(0)
                    for ti in range(NT):
                        ss_next = s_group(ti + 1) if ti + 1 < NT else None
                        pt = att.tile([P, CH], F32R, tag="pt", bufs=3)
                        # masked softmax numerator: exp(score*scale + mbias)
                        nc.scalar.activation(out=pt, in_=ss_cur, func=EXP,
                                             scale=SCALE,
                                             bias=mbias[:, ti:ti + 1])
                        for d in range(KD):
                            nc.tensor.matmul(
                                opsum[d],
                                vA[:, ti * VW + d * P: ti * VW + (d + 1) * P],
                                pt, start=(ti == 0), stop=(ti == NT - 1))
                        if ti == 0:
                            nc.vector.tensor_copy(out=den128, in_=pt)
                        else:
                            nc.vector.tensor_add(den128, den128, pt)
                        ss_cur = ss_next

                    # denominator: den[s] = column sum of den128 (P^T already
                    # masked by the EXP bias)
                    dps = apsum.tile([1, CH], F32, tag="bc", name="dps")
                    nc.tensor.matmul(dps, onec, den128, start=True, stop=True)

                    def drain_opsum():
                        osb = []
                        for d in range(KD):
                            ot = att.tile([P, CH], F32, tag=f"osb{d}",
                                          name=f"osb{d}")
                            nc.vector.tensor_copy(out=ot, in_=opsum[d])
                            osb.append(ot)
                        return osb

                    # mid-kernel: drain opsum banks via DVE FIRST so PE can
                    # reuse them for the next chunk without waiting on the
                    # reciprocal chain. Last chunk: reciprocal FIRST -- there
                    # is no next chunk, and the drains sitting ahead of it in
                    # the DVE queue would add ~2.4us to the exposed tail.
                    last = (sc == SH // CH - 1)
                    if not last:
                        osb = drain_opsum()
                    rec = att.tile([1, CH], F32, tag="rec")
                    nc.vector.reciprocal_approx_fast(out=rec, in_=dps)
                    recr = att.tile([1, CH], F32R, tag="recr")
                    nc.vector.tensor_copy(out=recr, in_=rec)
                    if last:
                        osb = drain_opsum()
                    bps = apsum.tile([P, CH], F32, tag="bc", name="bps")
                    nc.tensor.matmul(bps, ones, recr, start=True, stop=True)
                    bsb = att.tile([P, CH], F32, tag="bsb")
                    nc.vector.tensor_copy(out=bsb, in_=bps)
                    for d in range(KD):
                        fin = att.tile([P, CH], F32, tag=f"fin{d % 2}",
                                       name=f"fin{d}", bufs=2)
                        meng = nc.vector if d % 2 == 0 else nc.gpsimd
                        meng.tensor_mul(fin, osb[d], bsb)
                        eng = nc.sync if d % 2 == 0 else nc.gpsimd
                        eng.dma_start(
                            out=outT[d * P:(d + 1) * P, sc * CH:(sc + 1) * CH],
                            in_=fin)

    nc.compile()
    return nc


def kernel(x, mask, Wk, Wq, Wv):
    if "nc" not in _CACHE:
        _CACHE["nc"] = _build()
    nc = _CACHE["nc"]

    x = np.asarray(x, dtype=np.float32)
    mask_f = np.asarray(mask).astype(np.float32)
    wqT = np.ascontiguousarray(np.asarray(Wq, dtype=np.float32).T)
    wkT = np.ascontiguousarray(np.asarray(Wk, dtype=np.float32).T)
    wvT = np.ascontiguousarray(np.asarray(Wv, dtype=np.float32).T)

    in_maps = []
    xTs = [np.ascontiguousarray(x[b].T) for b in range(B)]
    mks = [np.ascontiguousarray(mask_f[b].reshape(NT, P).T) for b in range(B)]
    for b in range(B):
        for h in range(2):
            in_maps.append({
                "xT": xTs[b],
                "xsT": np.ascontiguousarray(xTs[b][:, h * SH:(h + 1) * SH]),
                "wqT": wqT, "wkT": wkT, "wvT": wvT,
                "maskT": mks[b],
            })

    res = run_bass_kernel_spmd(nc, in_maps, core_ids=list(range(8)))

    out = np.empty((B, S, D), dtype=np.float32)
    for b in range(B):
        for h in range(2):
            out[b, h * SH:(h + 1) * SH, :] = res.results[2 * b + h]["outT"].T
    return out



# revision 4
# speedup vs baseline: 1.5927x; 1.5927x over previous
"""Trainium2 Bass kernel for nn_AttentionHead (B=4, S=4096, D=512).

reference:
    K = x @ Wk.T; Q = x @ Wq.T; V = x @ Wv.T            # [B,S,D]
    scores[b,s,t] = <K[b,s], Q[b,t]> / sqrt(D)
    scores[b,:,t] = -1e12 where mask[b,t]==0
    out = softmax(scores, axis=t) @ V                    # [B,S,D]

Sharding: 8 cores = 4 batches x 2 sequence halves (rows s of the score
matrix). No collectives; each core computes Q^T/V for the full sequence of
its batch and K^T for its s-half only. (A pairwise AllGather variant that
deduplicates the Q/V projections was measured 230us SLOWER.)

Key optimizations over the straightforward version:

1. Mask compaction (host-side, pure gather): masked key positions t get
   weight exactly 0 after softmax (exp(-1e12*scale) == 0), so their Q/V
   columns and score columns are dead work. The host gathers only the
   ~50% surviving t columns of x^T (zero-padding to a multiple of 256)
   and the kernel sizes its Q/V projections and the score/AV loops to the
   compacted width SKP. Padded slots get mask bias -1e9 inside the EXP so
   they contribute exactly 0, identical to the reference semantics.

2. fp8 (e4m3) DoubleRow matmuls for the two big GEMMs (scores = Q^T.T@K^T
   and out^T += V.T@P^T): DoubleRow contracts 256 rows per pass (operands
   viewed [128, 2, free]) at 2x the bf16/f32r MAC rate. K^T/Q^T/V are
   produced in f32r PSUM by the projection matmuls and quantized to fp8
   on the ACT-engine PSUM->SBUF copy; P^T is quantized by the EXP
   activation itself. The softmax denominator is accumulated on the PE
   via a ones-column DoubleRow matmul over the same fp8 P^T used in the
   numerator (so quantization error largely cancels in the ratio).
   Projections stay f32r (full PE rate at N>=256).

Masking: mbias[t] = (mask[t]-1)*1e9 is added inside the EXP, so masked/
padded keys underflow to exactly 0 -- identical to the reference's -1e12
fill followed by softmax (requires >=1 unmasked key per batch, which
random 0/1 masks over 4096 positions guarantee).

Host passes x^T / W^T layouts and the t-gather (pure permutations/
selection; all FLOPs stay on device). The f32r DRAM declaration lets raw
fp32 bits feed f32r matmuls directly.
"""

import numpy as np

import concourse.bacc as bacc
import concourse.mybir as mybir
from concourse.bass_utils import run_bass_kernel_spmd
from concourse.tile import TileContext

B, S, D = 4, 4096, 512
SH = S // 2          # per-core s rows (half sequence)
P = 128              # partition tile
CH = 512             # free-dim chunk
KD = D // P          # 4 contraction tiles over d
SCALE = 1.0 / float(np.sqrt(D))

F32 = mybir.dt.float32
F32R = mybir.dt.float32r
F8 = mybir.dt.float8e4
COPY = mybir.ActivationFunctionType.Copy
EXP = mybir.ActivationFunctionType.Exp
DR = mybir.MatmulPerfMode.DoubleRow

FP8 = False           # fp8 DoubleRow for scores/AV (f32r fallback if False)

_CACHE = {}


def _build(skp, fp8):
    ntk = skp // P       # t-tiles over compacted keys
    npair = ntk // 2     # DoubleRow t-tile pairs
    dt_att = F8 if fp8 else F32R

    nc = bacc.Bacc(num_devices=8)
    xsT = nc.declare_dram_parameter("xsT", [D, SH], F32R, isOutput=False)
    xkT = nc.declare_dram_parameter("xkT", [D, skp], F32R, isOutput=False)
    wqT = nc.declare_dram_parameter("wqT", [D, D], F32R, isOutput=False)
    wkT = nc.declare_dram_parameter("wkT", [D, D], F32R, isOutput=False)
    wvT = nc.declare_dram_parameter("wvT", [D, D], F32R, isOutput=False)
    maskT = nc.declare_dram_parameter("maskT", [P, ntk], F32, isOutput=False)
    outT = nc.declare_dram_parameter("outT", [D, SH], F32, isOutput=True)

    # Q-projection chunks over the compacted width (last may be 256)
    qchunks = []
    c0 = 0
    while c0 < skp:
        w = min(CH, skp - c0)
        qchunks.append((c0, w))
        c0 += w

    with TileContext(nc) as tc:
        with tc.tile_pool(name="pers", bufs=1) as pers:
            qT = pers.tile([P, KD, skp], dt_att)     # [d-par, d-tile, t]
            kT = pers.tile([P, KD, SH], dt_att)      # [d-par, d-tile, s]
            vA = pers.tile([P, ntk, D], dt_att)      # [t-par, t-tile, d]
            mk = pers.tile([P, ntk], F32)
            ones = pers.tile([1, P], F32R)
            ones32 = pers.tile([1, P], F32)
            onec2 = pers.tile([P, 2, 1], dt_att)     # den ones (fp8 path)
            onec = pers.tile([P, 1], F32R)           # den ones (f32r path)
            onec32 = pers.tile([P, 2], F32)
            mbias = pers.tile([P, ntk], F32)

            # ---------------- phase 1: projections ----------------
            with tc.tile_pool(name="stage", bufs=1) as stage, \
                 tc.tile_pool(name="ppsum", bufs=3, space="PSUM") as ppsum:
                wq = stage.tile([P, KD * D], F32R, tag="wq")
                wk = stage.tile([P, KD * D], F32R, tag="wk")
                wv = stage.tile([P, KD * D], F32R, tag="wv")
                # PE warm-up: dummy matmuls into a trash PSUM bank while the
                # first DMAs are in flight -- keeps the HAM clock-gate at
                # 2.4GHz so the real matmuls start warm instead of paying
                # the ~3.4us half-rate ramp
                warm32 = stage.tile([P, CH], F32, tag="warm32")
                warm = stage.tile([P, CH], F32R, tag="warm")
                nc.vector.memset(warm32, 0.0)
                nc.vector.tensor_copy(out=warm, in_=warm32)
                for r in range(24):
                    wps = ppsum.tile([P, CH], F32, tag="warm", bufs=2,
                                     name="wps")
                    nc.tensor.matmul(wps, warm[:, 0:P], warm,
                                     start=True, stop=True)

                engs = [nc.sync, nc.gpsimd, nc.scalar]
                xr0 = stage.tile([P, KD * CH], F32R, tag="xr", bufs=2,
                                 name="xr0")
                # pair each k-tile's (wk, x) DMAs on ONE queue: the first K
                # matmul of k-tile j then needs a single queue-sem wait and
                # can start as soon as ITS pair lands, not after all eight
                for j in range(KD):
                    eng = engs[j % 3]
                    eng.dma_start(
                        out=wk[:, j * D:(j + 1) * D],
                        in_=wkT[j * P:(j + 1) * P, :])
                    eng.dma_start(
                        out=xr0[:, j * CH:(j + 1) * CH],
                        in_=xsT[j * P:(j + 1) * P, 0:CH])
                for j in range(KD):
                    nc.sync.dma_start(out=wq[:, j * D:(j + 1) * D],
                                      in_=wqT[j * P:(j + 1) * P, :])
                    nc.gpsimd.dma_start(out=wv[:, j * D:(j + 1) * D],
                                        in_=wvT[j * P:(j + 1) * P, :])

                # constants + mask bias (off the first-wave critical path)
                nc.scalar.dma_start(out=mk, in_=maskT[:, :])
                nc.vector.memset(ones32, 1.0)
                nc.vector.tensor_copy(out=ones, in_=ones32)
                nc.vector.memset(onec32, 1.0)
                nc.vector.tensor_copy(out=onec2[:, :, 0], in_=onec32)
                nc.vector.tensor_copy(out=onec, in_=onec32[:, 0:1])
                # mbias[p, i] = (mask-1)*1e9: 0 where kept, -1e9 where
                # masked/padded; exp(score*scale + mbias) underflows to 0
                nc.vector.tensor_scalar(mbias, mk, -1.0, 1.0e9,
                                        mybir.AluOpType.add,
                                        mybir.AluOpType.mult)
                if fp8:
                    # TRN e4m3 overflows to Inf above 240; scores*scale are
                    # ~N(0,1) so max exp over ~33M draws is ~e^5.9 > 240.
                    # Shift the exponent by -3: numerator and denominator
                    # both scale by e^-3, the softmax ratio is unchanged,
                    # and max P^T becomes ~e^2.9 = 18 << 240. (Weights
                    # below the fp8 subnormal floor flush to 0; their
                    # relative contribution is < 1e-3.)
                    nc.vector.tensor_scalar_add(mbias, mbias, -3.0)

                # K^T first (phase 2's first score groups need it earliest)
                for c in range(SH // CH):
                    if c == 0:
                        xr = xr0
                    else:
                        xr = stage.tile([P, KD * CH], F32R, tag="xr", bufs=2,
                                        name="xrk")
                        for j in range(KD):
                            nc.sync.dma_start(
                                out=xr[:, j * CH:(j + 1) * CH],
                                in_=xsT[j * P:(j + 1) * P, c * CH:(c + 1) * CH])
                    for jo in range(KD):
                        pq = ppsum.tile([P, CH], F32, tag="pq", name="pqk")
                        for kd in range(KD):
                            nc.tensor.matmul(
                                pq,
                                wk[:, kd * D + jo * P: kd * D + (jo + 1) * P],
                                xr[:, kd * CH:(kd + 1) * CH],
                                start=(kd == 0), stop=(kd == KD - 1))
                        nc.scalar.activation(
                            out=kT[:, jo, c * CH:(c + 1) * CH],
                            in_=pq, func=COPY)

                # Q^T and V from the compacted x^T, chunk by chunk
                for (c0, w) in qchunks:
                    xr = stage.tile([P, KD * CH], F32R, tag="xr", bufs=2,
                                    name="xrq")
                    for j in range(KD):
                        nc.sync.dma_start(
                            out=xr[:, j * CH:j * CH + w],
                            in_=xkT[j * P:(j + 1) * P, c0:c0 + w])
                    for jo in range(KD):
                        pq = ppsum.tile([P, CH], F32, tag="pq")
                        for kd in range(KD):
                            nc.tensor.matmul(
                                pq[:, 0:w],
                                wq[:, kd * D + jo * P: kd * D + (jo + 1) * P],
                                xr[:, kd * CH:kd * CH + w],
                                start=(kd == 0), stop=(kd == KD - 1))
                        nc.scalar.activation(
                            out=qT[:, jo, c0:c0 + w],
                            in_=pq[:, 0:w], func=COPY)
                    for tt in range(w // P):
                        ti = c0 // P + tt
                        pv = ppsum.tile([P, D], F32, tag="pv")
                        for kd in range(KD):
                            nc.tensor.matmul(
                                pv,
                                xr[:, kd * CH + tt * P: kd * CH + (tt + 1) * P],
                                wv[:, kd * D:(kd + 1) * D],
                                start=(kd == 0), stop=(kd == KD - 1))
                        nc.scalar.activation(
                            out=vA[:, ti, :], in_=pv, func=COPY)

            # ---------------- phase 2: attention ----------------
            with tc.tile_pool(name="att", bufs=1) as att, \
                 tc.tile_pool(name="apsum", bufs=1, space="PSUM") as apsum:

                for sc in range(SH // CH):
                    opsum = [apsum.tile([P, CH], F32, tag=f"o{d}",
                                        name=f"opsum{d}")
                             for d in range(KD)]
                    den128 = att.tile([P, CH], F32R, tag="den128")
                    dps = apsum.tile([1, CH], F32, tag="bc", name="dps")

                    if fp8:
                        # DoubleRow fp8: per t-pair, two score tiles ->
                        # EXP into the paired P^T buffer -> 4 AV matmuls
                        # + 1 den matmul, all contracting 256 t at once.
                        for u in range(npair):
                            pt = att.tile([P, 2, CH], F8, tag="pt", bufs=3)
                            for i in range(2):
                                ti = 2 * u + i
                                ss = apsum.tile([P, CH], F32, tag="s",
                                                bufs=3)
                                for j in range(KD // 2):
                                    nc.tensor.matmul(
                                        ss,
                                        qT[:, 2 * j:2 * j + 2,
                                           ti * P:(ti + 1) * P],
                                        kT[:, 2 * j:2 * j + 2,
                                           sc * CH:(sc + 1) * CH],
                                        start=(j == 0), stop=(j == 1),
                                        perf_mode=DR)
                                nc.scalar.activation(
                                    out=pt[:, i, :], in_=ss, func=EXP,
                                    scale=SCALE, bias=mbias[:, ti:ti + 1])
                            for d in range(KD):
                                nc.tensor.matmul(
                                    opsum[d],
                                    vA[:, 2 * u:2 * u + 2,
                                       d * P:(d + 1) * P],
                                    pt, start=(u == 0),
                                    stop=(u == npair - 1), perf_mode=DR)
                            nc.tensor.matmul(
                                dps, onec2, pt, start=(u == 0),
                                stop=(u == npair - 1), perf_mode=DR)
                    else:
                        # f32r path: per t-tile scores (4 matmuls over d),
                        # EXP, 4 AV matmuls; denominator accumulated on
                        # DVE then column-summed by a ones matmul.
                        def s_group(ti, sc=sc):
                            ss = apsum.tile([P, CH], F32, tag="s", bufs=3)
                            for kd in range(KD):
                                nc.tensor.matmul(
                                    ss,
                                    qT[:, kd, ti * P:(ti + 1) * P],
                                    kT[:, kd, sc * CH:(sc + 1) * CH],
                                    start=(kd == 0), stop=(kd == KD - 1))
                            return ss

                        ss_cur = s_group(0)
                        for ti in range(ntk):
                            ss_next = s_group(ti + 1) if ti + 1 < ntk else None
                            pt = att.tile([P, CH], F32R, tag="pt", bufs=3)
                            nc.scalar.activation(out=pt, in_=ss_cur,
                                                 func=EXP, scale=SCALE,
                                                 bias=mbias[:, ti:ti + 1])
                            for d in range(KD):
                                nc.tensor.matmul(
                                    opsum[d],
                                    vA[:, ti, d * P:(d + 1) * P],
                                    pt, start=(ti == 0),
                                    stop=(ti == ntk - 1))
                            if ti == 0:
                                nc.vector.tensor_copy(out=den128, in_=pt)
                            else:
                                nc.vector.tensor_add(den128, den128, pt)
                            ss_cur = ss_next
                        nc.tensor.matmul(dps, onec, den128,
                                         start=True, stop=True)

                    def drain_opsum():
                        osb = []
                        for d in range(KD):
                            ot = att.tile([P, CH], F32, tag=f"osb{d}",
                                          name=f"osb{d}")
                            nc.vector.tensor_copy(out=ot, in_=opsum[d])
                            osb.append(ot)
                        return osb

                    # mid-kernel: drain opsum banks via DVE FIRST so PE can
                    # reuse them for the next chunk without waiting on the
                    # reciprocal chain. Last chunk: reciprocal FIRST -- there
                    # is no next chunk, and the drains sitting ahead of it in
                    # the DVE queue would add ~2.4us to the exposed tail.
                    last = (sc == SH // CH - 1)
                    if not last:
                        osb = drain_opsum()
                    rec = att.tile([1, CH], F32, tag="rec")
                    nc.vector.reciprocal_approx_fast(out=rec, in_=dps)
                    recr = att.tile([1, CH], F32R, tag="recr")
                    nc.vector.tensor_copy(out=recr, in_=rec)
                    if last:
                        osb = drain_opsum()
                    bps = apsum.tile([P, CH], F32, tag="bc", name="bps")
                    nc.tensor.matmul(bps, ones, recr, start=True, stop=True)
                    bsb = att.tile([P, CH], F32, tag="bsb")
                    nc.vector.tensor_copy(out=bsb, in_=bps)
                    for d in range(KD):
                        fin = att.tile([P, CH], F32, tag=f"fin{d % 2}",
                                       name=f"fin{d}", bufs=2)
                        meng = nc.vector if d % 2 == 0 else nc.gpsimd
                        meng.tensor_mul(fin, osb[d], bsb)
                        eng = nc.sync if d % 2 == 0 else nc.gpsimd
                        eng.dma_start(
                            out=outT[d * P:(d + 1) * P, sc * CH:(sc + 1) * CH],
                            in_=fin)

    nc.compile()
    return nc


def _prep(x, mask, Wk, Wq, Wv):
    """Host-side layout prep: transposes + mask-compaction gather.
    Returns (skp, in_maps)."""
    x = np.asarray(x, dtype=np.float32)
    mask_np = np.asarray(mask)
    wqT = np.ascontiguousarray(np.asarray(Wq, dtype=np.float32).T)
    wkT = np.ascontiguousarray(np.asarray(Wk, dtype=np.float32).T)
    wvT = np.ascontiguousarray(np.asarray(Wv, dtype=np.float32).T)

    idxs = [np.nonzero(mask_np[b])[0] for b in range(B)]
    nk_max = max(len(ix) for ix in idxs)
    skp = max(256, ((nk_max + 255) // 256) * 256)
    ntk = skp // P

    in_maps = []
    for b in range(B):
        xT = np.ascontiguousarray(x[b].T)                  # [D, S]
        xk = np.zeros((D, skp), dtype=np.float32)
        xk[:, :len(idxs[b])] = xT[:, idxs[b]]
        mg = np.zeros(skp, dtype=np.float32)
        mg[:len(idxs[b])] = 1.0
        mkT = np.ascontiguousarray(mg.reshape(ntk, P).T)   # [P, ntk]
        for h in range(2):
            in_maps.append({
                "xsT": np.ascontiguousarray(xT[:, h * SH:(h + 1) * SH]),
                "xkT": xk,
                "wqT": wqT, "wkT": wkT, "wvT": wvT,
                "maskT": mkT,
            })
    return skp, in_maps


def _get_nc(skp):
    key = (skp, FP8)
    if key not in _CACHE:
        _CACHE[key] = _build(skp, FP8)
    return _CACHE[key]


def kernel(x, mask, Wk, Wq, Wv):
    skp, in_maps = _prep(x, mask, Wk, Wq, Wv)
    nc = _get_nc(skp)

    res = run_bass_kernel_spmd(nc, in_maps, core_ids=list(range(8)))

    out = np.empty((B, S, D), dtype=np.float32)
    for b in range(B):
        for h in range(2):
            out[b, h * SH:(h + 1) * SH, :] = res.results[2 * b + h]["outT"].T
    return out


# revision 31
# speedup vs baseline: 1.6859x; 1.0586x over previous
"""Trainium2 Bass kernel for nn_AttentionHead (B=4, S=4096, D=512).

reference:
    K = x @ Wk.T; Q = x @ Wq.T; V = x @ Wv.T            # [B,S,D]
    scores[b,s,t] = <K[b,s], Q[b,t]> / sqrt(D)
    scores[b,:,t] = -1e12 where mask[b,t]==0
    out = softmax(scores, axis=t) @ V                    # [B,S,D]

Sharding: 8 cores = 4 batches x 2 sequence halves (rows s of the score
matrix). No collectives; each core computes Q^T/V for the full sequence of
its batch and K^T for its s-half only. (A pairwise AllGather variant that
deduplicates the Q/V projections was measured 230us SLOWER.)

Key optimizations over the straightforward version:

1. Mask compaction (host-side, pure gather): masked key positions t get
   weight exactly 0 after softmax (exp(-1e12*scale) == 0), so their Q/V
   columns and score columns are dead work. The host gathers only the
   ~50% surviving t columns of x^T (zero-padding to a multiple of 256)
   and the kernel sizes its Q/V projections and the score/AV loops to the
   compacted width SKP. Padded slots get mask bias -1e9 inside the EXP so
   they contribute exactly 0, identical to the reference semantics.

2. fp8 (e4m3) DoubleRow matmuls for the two big GEMMs (scores = Q^T.T@K^T
   and out^T += V.T@P^T): DoubleRow contracts 256 rows per pass (operands
   viewed [128, 2, free]) at 2x the bf16/f32r MAC rate. K^T/Q^T/V are
   produced in f32r PSUM by the projection matmuls and quantized to fp8
   on the ACT-engine PSUM->SBUF copy; P^T is quantized by the EXP
   activation itself. The softmax denominator is accumulated on the PE
   via a ones-column DoubleRow matmul over the same fp8 P^T used in the
   numerator (so quantization error largely cancels in the ratio).
   Projections stay f32r (full PE rate at N>=256).

Masking: mbias[t] = (mask[t]-1)*1e9 is added inside the EXP, so masked/
padded keys underflow to exactly 0 -- identical to the reference's -1e12
fill followed by softmax (requires >=1 unmasked key per batch, which
random 0/1 masks over 4096 positions guarantee).

Host passes x^T / W^T layouts and the t-gather (pure permutations/
selection; all FLOPs stay on device). The f32r DRAM declaration lets raw
fp32 bits feed f32r matmuls directly.
"""

import numpy as np

import concourse.bacc as bacc
import concourse.mybir as mybir
from concourse.bass_utils import run_bass_kernel_spmd
from concourse.tile import TileContext

B, S, D = 4, 4096, 512
SH = S // 2          # per-core s rows (half sequence)
P = 128              # partition tile
CH = 512             # free-dim chunk
KD = D // P          # 4 contraction tiles over d
SCALE = 1.0 / float(np.sqrt(D))

F32 = mybir.dt.float32
F32R = mybir.dt.float32r
F8 = mybir.dt.float8e4
F8E5 = mybir.dt.float8e5
BF16 = mybir.dt.bfloat16
COPY = mybir.ActivationFunctionType.Copy
EXP = mybir.ActivationFunctionType.Exp
DR = mybir.MatmulPerfMode.DoubleRow

FP8 = False           # fp8 DoubleRow for scores/AV (f32r fallback if False)

_CACHE = {}


def _build(skp, fp8):
    ntk = skp // P       # t-tiles over compacted keys
    npair = ntk // 2     # DoubleRow t-tile pairs
    # Phase-2 operands (K^T/Q^T/V/P^T) in bf16: same PE rate as f32r, but
    # 2x lighter LD_WEIGHTS (FWL applies to non-fp32 weights), half the
    # SBUF traffic, and the tiles are produced by ACT copies/EXP anyway so
    # the conversion is free. Walrus forbids mixing 32-bit and 16-bit
    # matmul inputs, so BOTH operands switch together; projections stay
    # f32r end-to-end. Measured max rel err 5.9e-3 (gate 2e-2).
    dt_p2 = F8 if fp8 else BF16

    nc = bacc.Bacc(num_devices=8)
    xsT = nc.declare_dram_parameter("xsT", [D, SH], F32R, isOutput=False)
    xkT = nc.declare_dram_parameter("xkT", [D, skp], F32R, isOutput=False)
    wqT = nc.declare_dram_parameter("wqT", [D, D], F32R, isOutput=False)
    wkT = nc.declare_dram_parameter("wkT", [D, D], F32R, isOutput=False)
    wvT = nc.declare_dram_parameter("wvT", [D, D], F32R, isOutput=False)
    maskT = nc.declare_dram_parameter("maskT", [P, ntk], F32, isOutput=False)
    outT = nc.declare_dram_parameter("outT", [D, SH], F32, isOutput=True)

    # Q-projection chunks over the compacted width (last may be 256)
    qchunks = []
    c0 = 0
    while c0 < skp:
        w = min(CH, skp - c0)
        qchunks.append((c0, w))
        c0 += w

    with TileContext(nc) as tc:
        with tc.tile_pool(name="pers", bufs=1) as pers:
            qT = pers.tile([P, KD, skp], dt_p2)      # [d-par, d-tile, t]
            kT = pers.tile([P, KD, SH], dt_p2)       # [d-par, d-tile, s]
            vA = pers.tile([P, ntk, D], dt_p2)       # [t-par, t-tile, d]
            mk = pers.tile([P, ntk], F32)
            ones = pers.tile([1, P], F32R)
            ones32 = pers.tile([1, P], F32)
            # den ones (fp8 path): the two 1.0 weights sit 16B apart so the
            # DoubleRow LD_WEIGHTS outer free-AP step is even + 16B-aligned
            # (s3_lw_dual_fp8_restrictions rejects step<16 for dual-fp8)
            onec2 = pers.tile([P, 2, 16], dt_p2)
            onec = pers.tile([P, 1], F32R)           # den ones (f32r path)
            onec32 = pers.tile([P, 2], F32)
            mbias = pers.tile([P, ntk], F32)

            # ---------------- phase 1: projections ----------------
            with tc.tile_pool(name="stage", bufs=1) as stage, \
                 tc.tile_pool(name="ppsum", bufs=3, space="PSUM") as ppsum:
                wq = stage.tile([P, KD * D], F32R, tag="wq")
                wk = stage.tile([P, KD * D], F32R, tag="wk")
                wv = stage.tile([P, KD * D], F32R, tag="wv")
                # PE warm-up: dummy matmuls into a trash PSUM bank while the
                # first DMAs are in flight -- keeps the HAM clock-gate at
                # 2.4GHz so the real matmuls start warm instead of paying
                # the ~3.4us half-rate ramp. Single memset producer (the old
                # f32->f32r copy chain delayed the first warm matmul ~1us).
                warm = stage.tile([P, CH], F32R, tag="warm")
                nc.vector.memset(warm.bitcast(F32), 0.0)
                for r in range(24):
                    wps = ppsum.tile([P, CH], F32, tag="warm", bufs=2,
                                     name="wps")
                    nc.tensor.matmul(wps, warm[:, 0:P], warm,
                                     start=True, stop=True)

                engs = [nc.sync, nc.gpsimd, nc.scalar]
                xr0 = stage.tile([P, KD * CH], F32R, tag="xr", bufs=3,
                                 name="xr0")
                # pair each k-tile's (wk, x) DMAs on ONE queue: the first K
                # matmul of k-tile j then needs a single queue-sem wait and
                # can start as soon as ITS pair lands, not after all eight
                for j in range(KD):
                    eng = engs[j % 3]
                    eng.dma_start(
                        out=wk[:, j * D:(j + 1) * D],
                        in_=wkT[j * P:(j + 1) * P, :])
                    eng.dma_start(
                        out=xr0[:, j * CH:(j + 1) * CH],
                        in_=xsT[j * P:(j + 1) * P, 0:CH])
                for j in range(KD):
                    nc.sync.dma_start(out=wq[:, j * D:(j + 1) * D],
                                      in_=wqT[j * P:(j + 1) * P, :])
                    nc.gpsimd.dma_start(out=wv[:, j * D:(j + 1) * D],
                                        in_=wvT[j * P:(j + 1) * P, :])

                # constants + mask bias (off the first-wave critical path)
                nc.scalar.dma_start(out=mk, in_=maskT[:, :])
                nc.vector.memset(ones32, 1.0)
                nc.vector.tensor_copy(out=ones, in_=ones32)
                nc.gpsimd.memset(onec32, 1.0)
                nc.gpsimd.tensor_copy(out=onec2[:, :, 0], in_=onec32)
                nc.gpsimd.tensor_copy(out=onec, in_=onec32[:, 0:1])
                # mbias[p, i] = (mask-1)*1e9: 0 where kept, -1e9 where
                # masked/padded; exp(score*scale + mbias) underflows to 0
                nc.vector.tensor_scalar(mbias, mk, -1.0, 1.0e9,
                                        mybir.AluOpType.add,
                                        mybir.AluOpType.mult)
                if fp8:
                    # TRN e4m3 overflows to Inf above 240; scores*scale are
                    # ~N(0,1) so max exp over ~33M draws is ~e^5.9 > 240.
                    # Shift the exponent by -3: numerator and denominator
                    # both scale by e^-3, the softmax ratio is unchanged,
                    # and max P^T becomes ~e^2.9 = 18 << 240. (Weights
                    # below the fp8 subnormal floor flush to 0; their
                    # relative contribution is < 1e-3.)
                    nc.vector.tensor_scalar_add(mbias, mbias, -3.0)

                # K^T first (phase 2's first score groups need it earliest).
                # x-chunk DMAs rotate across the three DMA-capable queues so
                # consecutive chunks land in parallel instead of serializing
                # on the sync queue.
                for c in range(SH // CH):
                    if c == 0:
                        xr = xr0
                    else:
                        xr = stage.tile([P, KD * CH], F32R, tag="xr", bufs=3,
                                        name="xrk")
                        for j in range(KD):
                            engs[(c + j) % 3].dma_start(
                                out=xr[:, j * CH:(j + 1) * CH],
                                in_=xsT[j * P:(j + 1) * P, c * CH:(c + 1) * CH])
                    for jo in range(KD):
                        pq = ppsum.tile([P, CH], F32, tag="pq", name="pqk")
                        for kd in range(KD):
                            nc.tensor.matmul(
                                pq,
                                wk[:, kd * D + jo * P: kd * D + (jo + 1) * P],
                                xr[:, kd * CH:(kd + 1) * CH],
                                start=(kd == 0), stop=(kd == KD - 1))
                        nc.scalar.activation(
                            out=kT[:, jo, c * CH:(c + 1) * CH],
                            in_=pq, func=COPY)

                # Q^T and V from the compacted x^T, chunk by chunk
                for ci, (c0, w) in enumerate(qchunks):
                    xr = stage.tile([P, KD * CH], F32R, tag="xr", bufs=3,
                                    name="xrq")
                    for j in range(KD):
                        engs[(ci + j) % 3].dma_start(
                            out=xr[:, j * CH:j * CH + w],
                            in_=xkT[j * P:(j + 1) * P, c0:c0 + w])
                    for jo in range(KD):
                        pq = ppsum.tile([P, CH], F32, tag="pq")
                        for kd in range(KD):
                            nc.tensor.matmul(
                                pq[:, 0:w],
                                wq[:, kd * D + jo * P: kd * D + (jo + 1) * P],
                                xr[:, kd * CH:kd * CH + w],
                                start=(kd == 0), stop=(kd == KD - 1))
                        nc.scalar.activation(
                            out=qT[:, jo, c0:c0 + w],
                            in_=pq[:, 0:w], func=COPY)
                    for tt in range(w // P):
                        ti = c0 // P + tt
                        pv = ppsum.tile([P, D], F32, tag="pv")
                        for kd in range(KD):
                            nc.tensor.matmul(
                                pv,
                                xr[:, kd * CH + tt * P: kd * CH + (tt + 1) * P],
                                wv[:, kd * D:(kd + 1) * D],
                                start=(kd == 0), stop=(kd == KD - 1))
                        nc.scalar.activation(
                            out=vA[:, ti, :], in_=pv, func=COPY)

            # ---------------- phase 2: attention ----------------
            with tc.tile_pool(name="att", bufs=1) as att, \
                 tc.tile_pool(name="apsum", bufs=1, space="PSUM") as apsum:

                for sc in range(SH // CH):
                    opsum = [apsum.tile([P, CH], F32, tag=f"o{d}",
                                        name=f"opsum{d}")
                             for d in range(KD)]
                    den128 = att.tile([P, CH], F32R, tag="den128")
                    dps = apsum.tile([1, CH], F32, tag="bc", name="dps")

                    if fp8:
                        # DoubleRow fp8: per t-pair, two score tiles ->
                        # EXP into the paired P^T buffer -> 4 AV matmuls
                        # + 1 den matmul, all contracting 256 t at once.
                        for u in range(npair):
                            # P^T in e5m2: scores reach z=8.7 (heavy-tailed
                            # row norms), so exp(z-3) can top e4m3's 240->Inf
                            # cliff; e5m2 tops out at 57344 and its coarser
                            # mantissa averages out over ~10^3 softmax terms.
                            pt = att.tile([P, 2, CH], F8E5, tag="pt", bufs=3)
                            for i in range(2):
                                ti = 2 * u + i
                                ss = apsum.tile([P, CH], F32, tag="s",
                                                bufs=3)
                                for j in range(KD // 2):
                                    nc.tensor.matmul(
                                        ss,
                                        qT[:, 2 * j:2 * j + 2,
                                           ti * P:(ti + 1) * P],
                                        kT[:, 2 * j:2 * j + 2,
                                           sc * CH:(sc + 1) * CH],
                                        start=(j == 0), stop=(j == 1),
                                        perf_mode=DR)
                                nc.scalar.activation(
                                    out=pt[:, i, :], in_=ss, func=EXP,
                                    scale=SCALE, bias=mbias[:, ti:ti + 1])
                            for d in range(KD):
                                nc.tensor.matmul(
                                    opsum[d],
                                    vA[:, 2 * u:2 * u + 2,
                                       d * P:(d + 1) * P],
                                    pt, start=(u == 0),
                                    stop=(u == npair - 1), perf_mode=DR)
                            nc.tensor.matmul(
                                dps, onec2[:, :, 0:1], pt, start=(u == 0),
                                stop=(u == npair - 1), perf_mode=DR)
                    else:
                        # f32r path: per t-tile scores (4 matmuls over d),
                        # EXP, 4 AV matmuls; denominator accumulated on
                        # DVE then column-summed by a ones matmul.
                        def s_group(ti, sc=sc):
                            ss = apsum.tile([P, CH], F32, tag="s", bufs=3)
                            for kd in range(KD):
                                nc.tensor.matmul(
                                    ss,
                                    qT[:, kd, ti * P:(ti + 1) * P],
                                    kT[:, kd, sc * CH:(sc + 1) * CH],
                                    start=(kd == 0), stop=(kd == KD - 1))
                            return ss

                        ss_cur = s_group(0)
                        for ti in range(ntk):
                            ss_next = s_group(ti + 1) if ti + 1 < ntk else None
                            pt = att.tile([P, CH], dt_p2, tag="pt", bufs=3)
                            nc.scalar.activation(out=pt, in_=ss_cur,
                                                 func=EXP, scale=SCALE,
                                                 bias=mbias[:, ti:ti + 1])
                            for d in range(KD):
                                nc.tensor.matmul(
                                    opsum[d],
                                    vA[:, ti, d * P:(d + 1) * P],
                                    pt, start=(ti == 0),
                                    stop=(ti == ntk - 1))
                            if ti == 0:
                                nc.vector.tensor_copy(out=den128, in_=pt)
                            else:
                                nc.vector.tensor_add(den128, den128, pt)
                            ss_cur = ss_next
                        nc.tensor.matmul(dps, onec, den128,
                                         start=True, stop=True)

                    def drain_opsum():
                        osb = []
                        for d in range(KD):
                            ot = att.tile([P, CH], F32, tag=f"osb{d}",
                                          name=f"osb{d}")
                            nc.vector.tensor_copy(out=ot, in_=opsum[d])
                            osb.append(ot)
                        return osb

                    # mid-kernel: drain opsum banks via DVE FIRST so PE can
                    # reuse them for the next chunk without waiting on the
                    # reciprocal chain. Last chunk: reciprocal FIRST -- there
                    # is no next chunk, and the drains sitting ahead of it in
                    # the DVE queue would add ~2.4us to the exposed tail.
                    last = (sc == SH // CH - 1)
                    if not last:
                        osb = drain_opsum()
                    rec = att.tile([1, CH], F32, tag="rec")
                    nc.vector.reciprocal_approx_fast(out=rec, in_=dps)
                    recr = att.tile([1, CH], F32R, tag="recr")
                    nc.vector.tensor_copy(out=recr, in_=rec)
                    if last:
                        osb = drain_opsum()
                    bps = apsum.tile([P, CH], F32, tag="bc", name="bps")
                    nc.tensor.matmul(bps, ones, recr, start=True, stop=True)
                    bsb = att.tile([P, CH], F32, tag="bsb")
                    nc.vector.tensor_copy(out=bsb, in_=bps)
                    # final scale + store: mults split over DVE/GpSimd, the
                    # four output DMAs over four different queues so the
                    # last chunk's store tail is ~4x shorter
                    dma_engs = [nc.sync, nc.gpsimd, nc.scalar, nc.sync]
                    for d in range(KD):
                        fin = att.tile([P, CH], F32, tag=f"fin{d % 2}",
                                       name=f"fin{d}", bufs=2)
                        meng = nc.vector if d % 2 == 0 else nc.gpsimd
                        meng.tensor_mul(fin, osb[d], bsb)
                        dma_engs[d].dma_start(
                            out=outT[d * P:(d + 1) * P, sc * CH:(sc + 1) * CH],
                            in_=fin)

    nc.compile()
    return nc


def _prep(x, mask, Wk, Wq, Wv):
    """Host-side layout prep: transposes + mask-compaction gather.
    Returns (skp, in_maps)."""
    x = np.asarray(x, dtype=np.float32)
    mask_np = np.asarray(mask)
    wqT = np.ascontiguousarray(np.asarray(Wq, dtype=np.float32).T)
    wkT = np.ascontiguousarray(np.asarray(Wk, dtype=np.float32).T)
    wvT = np.ascontiguousarray(np.asarray(Wv, dtype=np.float32).T)

    idxs = [np.nonzero(mask_np[b])[0] for b in range(B)]
    nk_max = max(len(ix) for ix in idxs)
    skp = max(256, ((nk_max + 127) // 128) * 128)
    if FP8:
        skp = max(256, ((nk_max + 255) // 256) * 256)
    ntk = skp // P

    in_maps = []
    for b in range(B):
        xT = np.ascontiguousarray(x[b].T)                  # [D, S]
        xk = np.zeros((D, skp), dtype=np.float32)
        xk[:, :len(idxs[b])] = xT[:, idxs[b]]
        mg = np.zeros(skp, dtype=np.float32)
        mg[:len(idxs[b])] = 1.0
        mkT = np.ascontiguousarray(mg.reshape(ntk, P).T)   # [P, ntk]
        for h in range(2):
            in_maps.append({
                "xsT": np.ascontiguousarray(xT[:, h * SH:(h + 1) * SH]),
                "xkT": xk,
                "wqT": wqT, "wkT": wkT, "wvT": wvT,
                "maskT": mkT,
            })
    return skp, in_maps


def _get_nc(skp):
    key = (skp, FP8)
    if key not in _CACHE:
        _CACHE[key] = _build(skp, FP8)
    return _CACHE[key]


def kernel(x, mask, Wk, Wq, Wv):
    skp, in_maps = _prep(x, mask, Wk, Wq, Wv)
    nc = _get_nc(skp)

    res = run_bass_kernel_spmd(nc, in_maps, core_ids=list(range(8)))

    out = np.empty((B, S, D), dtype=np.float32)
    for b in range(B):
        for h in range(2):
            out[b, h * SH:(h + 1) * SH, :] = res.results[2 * b + h]["outT"].T
    return out


# revision 32
# speedup vs baseline: 1.7801x; 1.0559x over previous
"""Trainium2 Bass kernel for nn_AttentionHead (B=4, S=4096, D=512).

reference:
    K = x @ Wk.T; Q = x @ Wq.T; V = x @ Wv.T            # [B,S,D]
    scores[b,s,t] = <K[b,s], Q[b,t]> / sqrt(D)
    scores[b,:,t] = -1e12 where mask[b,t]==0
    out = softmax(scores, axis=t) @ V                    # [B,S,D]

Sharding: 8 cores = 4 batches x 2 sequence halves (rows s of the score
matrix). No collectives; each core computes Q^T/V for the full compacted
sequence of its batch and K^T for its s-half only. (A pairwise AllGather
that deduplicates the Q/V projections measured 230us SLOWER at ~40GB/s
effective inter-core bandwidth.)

Optimizations (measured on HW, cumulative 354us -> ~200us):

1. Mask compaction (host-side, pure gather): masked key positions t get
   weight exactly 0 after softmax (exp(-1e12*scale) == 0 in fp32), so
   their Q/V columns and score columns are dead work. The host gathers
   the ~50% surviving t columns of x^T (zero-padded to a multiple of
   128) and all t loops run over the compacted width SKP. Padded slots
   get bias -1e9 inside the EXP so they contribute exactly 0.

2. bf16 everywhere on the PE: same 1 column/cycle rate as f32r, but the
   weight (stationary) loads use FWL (2x bandwidth, f32r gets none), so
   back-to-back matmuls run at the 518-cycle floor (216ns vs 230ns f32r,
   HW-measured). fp32->bf16 casts ride idle DVE cycles in phase 1;
   K^T/Q^T/V/P^T are quantized for free by the ACT-engine PSUM->SBUF
   copies / EXP. fp8 DoubleRow was tried and measured: score-side e4m3
   fails the 2e-2 gate outright (7e-2+, softmax argmax flips), and even
   AV-only e4m3 P/V gives 5e-2 on concentrated-softmax rows. Walrus
   also forbids mixing 32-bit and 16-bit matmul inputs, so bf16 applies
   to whole matmuls. Measured end-to-end err: 7.0e-3 (gate 2e-2).

3. Dataflow: K^T DMA+compute first so phase 2 can start earliest; x
   chunk DMAs ride only the sync/gpsimd queues (the scalar queue stalls
   DMA issues behind dependent ACT copies); each s-chunk's softmax
   epilogue (den/broadcast matmuls + reciprocal chain) is DELAYED into
   the next chunk's score stream so the PE never waits on the DVE
   reciprocal; final scale reads the broadcast PSUM directly.

Masking: mbias[t] = (mask[t]-1)*1e9 added inside the EXP; masked/padded
keys underflow to exactly 0 -- identical to the reference's -1e12 fill
followed by softmax (requires >=1 unmasked key per batch, which random
0/1 masks over 4096 positions guarantee).

Host passes x^T / W^T layouts and the t-gather (pure permutations /
selection; all FLOPs stay on device). The f32r DRAM declaration lets
raw fp32 bits feed the on-device bf16 casts directly.
"""

import numpy as np

import concourse.bacc as bacc
import concourse.mybir as mybir
from concourse.bass_utils import run_bass_kernel_spmd
from concourse.tile import TileContext

B, S, D = 4, 4096, 512
SH = S // 2          # per-core s rows (half sequence)
P = 128              # partition tile
CH = 512             # free-dim chunk
KD = D // P          # 4 contraction tiles over d
SCALE = 1.0 / float(np.sqrt(D))

F32 = mybir.dt.float32
F32R = mybir.dt.float32r
BF16 = mybir.dt.bfloat16
COPY = mybir.ActivationFunctionType.Copy
EXP = mybir.ActivationFunctionType.Exp

_CACHE = {}


def _build(skp):
    ntk = skp // P       # t-tiles over compacted keys

    nc = bacc.Bacc(num_devices=8)
    xsT = nc.declare_dram_parameter("xsT", [D, SH], F32R, isOutput=False)
    xkT = nc.declare_dram_parameter("xkT", [D, skp], F32R, isOutput=False)
    wqT = nc.declare_dram_parameter("wqT", [D, D], F32R, isOutput=False)
    wkT = nc.declare_dram_parameter("wkT", [D, D], F32R, isOutput=False)
    wvT = nc.declare_dram_parameter("wvT", [D, D], F32R, isOutput=False)
    maskT = nc.declare_dram_parameter("maskT", [P, ntk], F32, isOutput=False)
    outT = nc.declare_dram_parameter("outT", [D, SH], F32, isOutput=True)

    # Q/V-projection chunks over the compacted width (last may be short)
    qchunks = []
    c0 = 0
    while c0 < skp:
        w = min(CH, skp - c0)
        qchunks.append((c0, w))
        c0 += w

    with TileContext(nc) as tc:
        with tc.tile_pool(name="pers", bufs=1) as pers:
            qT = pers.tile([P, KD, skp], BF16)       # [d-par, d-tile, t]
            kT = pers.tile([P, KD, SH], BF16)        # [d-par, d-tile, s]
            vA = pers.tile([P, ntk, D], BF16)        # [t-par, t-tile, d]
            mk = pers.tile([P, ntk], F32)
            ones = pers.tile([1, P], F32R)
            ones32 = pers.tile([1, P], F32)
            onec = pers.tile([P, 1], F32R)           # den column-sum weights
            onec32 = pers.tile([P, 1], F32)
            mbias = pers.tile([P, ntk], F32)

            # ---------------- phase 1: projections ----------------
            with tc.tile_pool(name="stage", bufs=1) as stage, \
                 tc.tile_pool(name="ppsum", bufs=3, space="PSUM") as ppsum:
                wq32 = stage.tile([P, KD * D], F32R, tag="wq32")
                wk32 = stage.tile([P, KD * D], F32R, tag="wk32")
                wv32 = stage.tile([P, KD * D], F32R, tag="wv32")
                wq = stage.tile([P, KD * D], BF16, tag="wq")
                wk = stage.tile([P, KD * D], BF16, tag="wk")
                wv = stage.tile([P, KD * D], BF16, tag="wv")
                # PE warm-up: dummy matmuls into a trash PSUM bank while the
                # first DMAs are in flight -- keeps the HAM clock-gate at
                # 2.4GHz so the real matmuls start warm instead of paying
                # the ~3.4us half-rate ramp
                warm = stage.tile([P, CH], BF16, tag="warm")
                nc.vector.memset(warm, 0.0)
                for r in range(24):
                    wps = ppsum.tile([P, CH], F32, tag="warm", bufs=2,
                                     name="wps")
                    nc.tensor.matmul(wps, warm[:, 0:P], warm,
                                     start=True, stop=True)

                dmae = [nc.sync, nc.gpsimd]
                # K-path loads FIRST (wk + all xsT chunks): phase 1 start-up
                # is HBM-bound, so everything the K pipeline needs outranks
                # wq/wv/xk. Pair each k-tile's (wk, x) on one queue so the
                # first K matmul waits on a single queue-sem.
                xrk32 = []
                for c in range(SH // CH):
                    xrk32.append(stage.tile([P, KD * CH], F32R, tag="xr32",
                                            bufs=4, name=f"xrk32_{c}"))
                for j in range(KD):
                    eng = dmae[j % 2]
                    eng.dma_start(
                        out=wk32[:, j * D:(j + 1) * D],
                        in_=wkT[j * P:(j + 1) * P, :])
                    eng.dma_start(
                        out=xrk32[0][:, j * CH:(j + 1) * CH],
                        in_=xsT[j * P:(j + 1) * P, 0:CH])
                for j in range(KD):
                    nc.vector.tensor_copy(out=wk[:, j * D:(j + 1) * D],
                                          in_=wk32[:, j * D:(j + 1) * D])
                for c in range(1, SH // CH):
                    for j in range(KD):
                        dmae[(c + j) % 2].dma_start(
                            out=xrk32[c][:, j * CH:(j + 1) * CH],
                            in_=xsT[j * P:(j + 1) * P, c * CH:(c + 1) * CH])
                for j in range(KD):
                    nc.sync.dma_start(out=wq32[:, j * D:(j + 1) * D],
                                      in_=wqT[j * P:(j + 1) * P, :])
                    nc.gpsimd.dma_start(out=wv32[:, j * D:(j + 1) * D],
                                        in_=wvT[j * P:(j + 1) * P, :])

                # constants + mask bias (off the first-wave critical path)
                nc.scalar.dma_start(out=mk, in_=maskT[:, :])
                nc.vector.memset(ones32, 1.0)
                nc.vector.tensor_copy(out=ones, in_=ones32)
                nc.vector.memset(onec32, 1.0)
                nc.vector.tensor_copy(out=onec, in_=onec32)
                # mbias[p, i] = (mask-1)*1e9: 0 where kept, -1e9 where
                # masked/padded; exp(score*scale + mbias) underflows to 0
                nc.vector.tensor_scalar(mbias, mk, -1.0, 1.0e9,
                                        mybir.AluOpType.add,
                                        mybir.AluOpType.mult)

                def cast_chunk(xr32, w):
                    """fp32 -> bf16 x-chunk cast, per k-tile slice so the
                    first matmul only waits for its own slice."""
                    xr = stage.tile([P, KD * CH], BF16, tag="xr", bufs=3,
                                    name="xr")
                    for j in range(KD):
                        nc.vector.tensor_copy(
                            out=xr[:, j * CH:j * CH + w],
                            in_=xr32[:, j * CH:j * CH + w])
                    return xr

                # K^T first (phase 2's first score groups need it earliest)
                for c in range(SH // CH):
                    xr = cast_chunk(xrk32[c], CH)
                    for jo in range(KD):
                        pq = ppsum.tile([P, CH], F32, tag="pq", name="pqk")
                        for kd in range(KD):
                            nc.tensor.matmul(
                                pq,
                                wk[:, kd * D + jo * P: kd * D + (jo + 1) * P],
                                xr[:, kd * CH:(kd + 1) * CH],
                                start=(kd == 0), stop=(kd == KD - 1))
                        nc.scalar.activation(
                            out=kT[:, jo, c * CH:(c + 1) * CH],
                            in_=pq, func=COPY)
                    if c == 0:
                        for j in range(KD):
                            nc.vector.tensor_copy(
                                out=wq[:, j * D:(j + 1) * D],
                                in_=wq32[:, j * D:(j + 1) * D])
                        nc.vector.tensor_copy(out=wv, in_=wv32)

                # Q^T and V from the compacted x^T, chunk by chunk
                for ci, (c0, w) in enumerate(qchunks):
                    xr32 = stage.tile([P, KD * CH], F32R, tag="xr32", bufs=4,
                                      name="xrq32")
                    for j in range(KD):
                        dmae[(ci + j) % 2].dma_start(
                            out=xr32[:, j * CH:j * CH + w],
                            in_=xkT[j * P:(j + 1) * P, c0:c0 + w])
                    xr = cast_chunk(xr32, w)
                    for jo in range(KD):
                        pq = ppsum.tile([P, CH], F32, tag="pq")
                        for kd in range(KD):
                            nc.tensor.matmul(
                                pq[:, 0:w],
                                wq[:, kd * D + jo * P: kd * D + (jo + 1) * P],
                                xr[:, kd * CH:kd * CH + w],
                                start=(kd == 0), stop=(kd == KD - 1))
                        nc.scalar.activation(
                            out=qT[:, jo, c0:c0 + w],
                            in_=pq[:, 0:w], func=COPY)
                    for tt in range(w // P):
                        ti = c0 // P + tt
                        pv = ppsum.tile([P, D], F32, tag="pv")
                        for kd in range(KD):
                            nc.tensor.matmul(
                                pv,
                                xr[:, kd * CH + tt * P: kd * CH + (tt + 1) * P],
                                wv[:, kd * D:(kd + 1) * D],
                                start=(kd == 0), stop=(kd == KD - 1))
                        nc.scalar.activation(
                            out=vA[:, ti, :], in_=pv, func=COPY)

            # ---------------- phase 2: attention ----------------
            with tc.tile_pool(name="att", bufs=1) as att, \
                 tc.tile_pool(name="apsum", bufs=1, space="PSUM") as apsum:

                nchunk = SH // CH
                pending = [None]     # delayed epilogue from previous chunk

                def make_epilogue(den128, osb, sc):
                    def emit():
                        # den[s] = column sum of den128 (P^T already masked
                        # by the EXP bias), then out *= 1/den via a rank-1
                        # broadcast matmul; the scale mult reads the
                        # broadcast PSUM directly (no SBUF staging copy)
                        dps = apsum.tile([1, CH], F32, tag="bc", name="dps")
                        nc.tensor.matmul(dps, onec, den128,
                                         start=True, stop=True)
                        rec = att.tile([1, CH], F32, tag="rec")
                        nc.vector.reciprocal_approx_fast(out=rec, in_=dps)
                        recr = att.tile([1, CH], F32R, tag="recr")
                        nc.vector.tensor_copy(out=recr, in_=rec)
                        bps = apsum.tile([P, CH], F32, tag="bc", name="bps")
                        nc.tensor.matmul(bps, ones, recr,
                                         start=True, stop=True)
                        dma_engs = [nc.sync, nc.gpsimd, nc.scalar, nc.sync]
                        for d in range(KD):
                            fin = att.tile([P, CH], F32, tag=f"fin{d % 2}",
                                           name=f"fin{d}", bufs=2)
                            nc.vector.tensor_mul(fin, osb[d], bps)
                            dma_engs[d].dma_start(
                                out=outT[d * P:(d + 1) * P,
                                         sc * CH:(sc + 1) * CH],
                                in_=fin)
                    return emit

                for sc in range(nchunk):
                    opsum = [apsum.tile([P, CH], F32, tag=f"o{d}",
                                        name=f"opsum{d}")
                             for d in range(KD)]
                    # den128 accumulates P^T on the DVE (off the PE); bufs=2
                    # because the delayed den matmul still reads chunk sc's
                    # accumulator while chunk sc+1 starts a fresh one
                    den128 = att.tile([P, CH], F32R, tag="den128", bufs=2)

                    def s_group(ti, sc=sc):
                        ss = apsum.tile([P, CH], F32, tag="s", bufs=3)
                        for kd in range(KD):
                            nc.tensor.matmul(
                                ss,
                                qT[:, kd, ti * P:(ti + 1) * P],
                                kT[:, kd, sc * CH:(sc + 1) * CH],
                                start=(kd == 0), stop=(kd == KD - 1))
                        return ss

                    ss_cur = s_group(0)
                    for ti in range(ntk):
                        ss_next = s_group(ti + 1) if ti + 1 < ntk else None
                        pt = att.tile([P, CH], BF16, tag="pt", bufs=3)
                        # masked softmax numerator: exp(score*scale + mbias)
                        nc.scalar.activation(out=pt, in_=ss_cur, func=EXP,
                                             scale=SCALE,
                                             bias=mbias[:, ti:ti + 1])
                        for d in range(KD):
                            nc.tensor.matmul(
                                opsum[d],
                                vA[:, ti, d * P:(d + 1) * P],
                                pt, start=(ti == 0), stop=(ti == ntk - 1))
                        if ti == 0:
                            nc.vector.tensor_copy(out=den128, in_=pt)
                        else:
                            nc.vector.tensor_add(den128, den128, pt)
                        if ti == 2 and pending[0] is not None:
                            # previous chunk's epilogue: its den/broadcast
                            # matmuls slot in here so the PE never idles
                            # waiting on the DVE reciprocal chain
                            pending[0]()
                            pending[0] = None
                        ss_cur = ss_next

                    # drain the AV accumulators now (frees the PSUM banks
                    # for the next chunk); the rest of the epilogue waits
                    osb = []
                    for d in range(KD):
                        ot = att.tile([P, CH], F32, tag=f"osb{d}",
                                      name=f"osb{d}")
                        nc.vector.tensor_copy(out=ot, in_=opsum[d])
                        osb.append(ot)
                    pending[0] = make_epilogue(den128, osb, sc)

                pending[0]()

    nc.compile()
    return nc


def _prep(x, mask, Wk, Wq, Wv):
    """Host-side layout prep: transposes + mask-compaction gather.
    Returns (skp, in_maps)."""
    x = np.asarray(x, dtype=np.float32)
    mask_np = np.asarray(mask)
    wqT = np.ascontiguousarray(np.asarray(Wq, dtype=np.float32).T)
    wkT = np.ascontiguousarray(np.asarray(Wk, dtype=np.float32).T)
    wvT = np.ascontiguousarray(np.asarray(Wv, dtype=np.float32).T)

    idxs = [np.nonzero(mask_np[b])[0] for b in range(B)]
    nk_max = max(len(ix) for ix in idxs)
    skp = max(256, ((nk_max + 127) // 128) * 128)
    ntk = skp // P

    in_maps = []
    for b in range(B):
        xT = np.ascontiguousarray(x[b].T)                  # [D, S]
        xk = np.zeros((D, skp), dtype=np.float32)
        xk[:, :len(idxs[b])] = xT[:, idxs[b]]
        mg = np.zeros(skp, dtype=np.float32)
        mg[:len(idxs[b])] = 1.0
        mkT = np.ascontiguousarray(mg.reshape(ntk, P).T)   # [P, ntk]
        for h in range(2):
            in_maps.append({
                "xsT": np.ascontiguousarray(xT[:, h * SH:(h + 1) * SH]),
                "xkT": xk,
                "wqT": wqT, "wkT": wkT, "wvT": wvT,
                "maskT": mkT,
            })
    return skp, in_maps


def _get_nc(skp):
    if skp not in _CACHE:
        _CACHE[skp] = _build(skp)
    return _CACHE[skp]


def kernel(x, mask, Wk, Wq, Wv):
    skp, in_maps = _prep(x, mask, Wk, Wq, Wv)
    nc = _get_nc(skp)

    res = run_bass_kernel_spmd(nc, in_maps, core_ids=list(range(8)))

    out = np.empty((B, S, D), dtype=np.float32)
    for b in range(B):
        for h in range(2):
            out[b, h * SH:(h + 1) * SH, :] = res.results[2 * b + h]["outT"].T
    return out


# revision 34
# speedup vs baseline: 1.7930x; 1.0072x over previous
"""Trainium2 Bass kernel for nn_AttentionHead (B=4, S=4096, D=512).

reference:
    K = x @ Wk.T; Q = x @ Wq.T; V = x @ Wv.T            # [B,S,D]
    scores[b,s,t] = <K[b,s], Q[b,t]> / sqrt(D)
    scores[b,:,t] = -1e12 where mask[b,t]==0
    out = softmax(scores, axis=t) @ V                    # [B,S,D]

Sharding: 8 cores = 4 batches x 2 sequence halves (rows s of the score
matrix). No collectives; each core computes Q^T/V for the full compacted
sequence of its batch and K^T for its s-half only. (A pairwise AllGather
that deduplicates the Q/V projections measured 230us SLOWER at ~40GB/s
effective inter-core bandwidth.)

Optimizations (measured on HW, cumulative 354us -> ~200us):

1. Mask compaction (host-side, pure gather): masked key positions t get
   weight exactly 0 after softmax (exp(-1e12*scale) == 0 in fp32), so
   their Q/V columns and score columns are dead work. The host gathers
   the ~50% surviving t columns of x^T (zero-padded to a multiple of
   128) and all t loops run over the compacted width SKP. Padded slots
   get bias -1e9 inside the EXP so they contribute exactly 0.

2. bf16 everywhere on the PE: same 1 column/cycle rate as f32r, but the
   weight (stationary) loads use FWL (2x bandwidth, f32r gets none), so
   back-to-back matmuls run at the 518-cycle floor (216ns vs 230ns f32r,
   HW-measured). fp32->bf16 casts ride idle DVE cycles in phase 1;
   K^T/Q^T/V/P^T are quantized for free by the ACT-engine PSUM->SBUF
   copies / EXP. fp8 DoubleRow was tried and measured: score-side e4m3
   fails the 2e-2 gate outright (7e-2+, softmax argmax flips), and even
   AV-only e4m3 P/V gives 5e-2 on concentrated-softmax rows. Walrus
   also forbids mixing 32-bit and 16-bit matmul inputs, so bf16 applies
   to whole matmuls. Measured end-to-end err: 7.0e-3 (gate 2e-2).

3. Dataflow: K^T DMA+compute first so phase 2 can start earliest; x
   chunk DMAs ride only the sync/gpsimd queues (the scalar queue stalls
   DMA issues behind dependent ACT copies); each s-chunk's softmax
   epilogue (den/broadcast matmuls + reciprocal chain) is DELAYED into
   the next chunk's score stream so the PE never waits on the DVE
   reciprocal; final scale reads the broadcast PSUM directly.

Masking: mbias[t] = (mask[t]-1)*1e9 added inside the EXP; masked/padded
keys underflow to exactly 0 -- identical to the reference's -1e12 fill
followed by softmax (requires >=1 unmasked key per batch, which random
0/1 masks over 4096 positions guarantee).

Host passes x^T / W^T layouts and the t-gather (pure permutations /
selection; all FLOPs stay on device). The f32r DRAM declaration lets
raw fp32 bits feed the on-device bf16 casts directly.
"""

import numpy as np

import concourse.bacc as bacc
import concourse.mybir as mybir
from concourse.bass_utils import run_bass_kernel_spmd
from concourse.tile import TileContext

B, S, D = 4, 4096, 512
SH = S // 2          # per-core s rows (half sequence)
P = 128              # partition tile
CH = 512             # free-dim chunk
KD = D // P          # 4 contraction tiles over d
SCALE = 1.0 / float(np.sqrt(D))

F32 = mybir.dt.float32
F32R = mybir.dt.float32r
BF16 = mybir.dt.bfloat16
COPY = mybir.ActivationFunctionType.Copy
EXP = mybir.ActivationFunctionType.Exp

_CACHE = {}


def _build(skp):
    ntk = skp // P       # t-tiles over compacted keys

    nc = bacc.Bacc(num_devices=8)
    xsT = nc.declare_dram_parameter("xsT", [D, SH], F32R, isOutput=False)
    xkT = nc.declare_dram_parameter("xkT", [D, skp], F32R, isOutput=False)
    wqT = nc.declare_dram_parameter("wqT", [D, D], F32R, isOutput=False)
    wkT = nc.declare_dram_parameter("wkT", [D, D], F32R, isOutput=False)
    wvT = nc.declare_dram_parameter("wvT", [D, D], F32R, isOutput=False)
    maskT = nc.declare_dram_parameter("maskT", [P, ntk], F32, isOutput=False)
    outT = nc.declare_dram_parameter("outT", [D, SH], F32, isOutput=True)

    # Q/V-projection chunks over the compacted width (last may be short)
    qchunks = []
    c0 = 0
    while c0 < skp:
        w = min(CH, skp - c0)
        qchunks.append((c0, w))
        c0 += w

    with TileContext(nc) as tc:
        with tc.tile_pool(name="pers", bufs=1) as pers:
            qT = pers.tile([P, KD, skp], BF16)       # [d-par, d-tile, t]
            kT = pers.tile([P, KD, SH], BF16)        # [d-par, d-tile, s]
            vA = pers.tile([P, ntk, D], BF16)        # [t-par, t-tile, d]
            mk = pers.tile([P, ntk], F32)
            ones = pers.tile([1, P], F32R)
            ones32 = pers.tile([1, P], F32)
            onec = pers.tile([P, 1], F32R)           # den column-sum weights
            onec32 = pers.tile([P, 1], F32)
            mbias = pers.tile([P, ntk], F32)

            # ---------------- phase 1: projections ----------------
            with tc.tile_pool(name="stage", bufs=1) as stage, \
                 tc.tile_pool(name="ppsum", bufs=3, space="PSUM") as ppsum:
                wq32 = stage.tile([P, KD * D], F32R, tag="wq32")
                wk32 = stage.tile([P, KD * D], F32R, tag="wk32")
                wv32 = stage.tile([P, KD * D], F32R, tag="wv32")
                wq = stage.tile([P, KD * D], BF16, tag="wq")
                wk = stage.tile([P, KD * D], BF16, tag="wk")
                wv = stage.tile([P, KD * D], BF16, tag="wv")
                # PE warm-up: dummy matmuls into a trash PSUM bank while the
                # first DMAs are in flight -- keeps the HAM clock-gate at
                # 2.4GHz so the real matmuls start warm instead of paying
                # the ~3.4us half-rate ramp
                warm = stage.tile([P, CH], BF16, tag="warm")
                nc.vector.memset(warm, 0.0)
                for r in range(24):
                    wps = ppsum.tile([P, CH], F32, tag="warm", bufs=2,
                                     name="wps")
                    nc.tensor.matmul(wps, warm[:, 0:P], warm,
                                     start=True, stop=True)

                dmae = [nc.sync, nc.gpsimd]
                # K-path loads FIRST (wk + all xsT chunks): phase 1 start-up
                # is HBM-bound, so everything the K pipeline needs outranks
                # wq/wv/xk. Pair each k-tile's (wk, x) on one queue so the
                # first K matmul waits on a single queue-sem.
                xrk32 = []
                for c in range(SH // CH):
                    xrk32.append(stage.tile([P, KD * CH], F32R, tag="xr32",
                                            bufs=4, name=f"xrk32_{c}"))
                for j in range(KD):
                    eng = dmae[j % 2]
                    eng.dma_start(
                        out=wk32[:, j * D:(j + 1) * D],
                        in_=wkT[j * P:(j + 1) * P, :])
                    eng.dma_start(
                        out=xrk32[0][:, j * CH:(j + 1) * CH],
                        in_=xsT[j * P:(j + 1) * P, 0:CH])
                for j in range(KD):
                    nc.vector.tensor_copy(out=wk[:, j * D:(j + 1) * D],
                                          in_=wk32[:, j * D:(j + 1) * D])
                for c in range(1, SH // CH):
                    for j in range(KD):
                        dmae[(c + j) % 2].dma_start(
                            out=xrk32[c][:, j * CH:(j + 1) * CH],
                            in_=xsT[j * P:(j + 1) * P, c * CH:(c + 1) * CH])
                for j in range(KD):
                    nc.sync.dma_start(out=wq32[:, j * D:(j + 1) * D],
                                      in_=wqT[j * P:(j + 1) * P, :])
                    nc.gpsimd.dma_start(out=wv32[:, j * D:(j + 1) * D],
                                        in_=wvT[j * P:(j + 1) * P, :])

                # constants + mask bias (off the first-wave critical path)
                nc.scalar.dma_start(out=mk, in_=maskT[:, :])
                nc.vector.memset(ones32, 1.0)
                nc.vector.tensor_copy(out=ones, in_=ones32)
                nc.vector.memset(onec32, 1.0)
                nc.vector.tensor_copy(out=onec, in_=onec32)
                # mbias[p, i] = (mask-1)*1e9: 0 where kept, -1e9 where
                # masked/padded; exp(score*scale + mbias) underflows to 0
                nc.vector.tensor_scalar(mbias, mk, -1.0, 1.0e9,
                                        mybir.AluOpType.add,
                                        mybir.AluOpType.mult)

                def cast_chunk(xr32, w):
                    """fp32 -> bf16 x-chunk cast, per k-tile slice so the
                    first matmul only waits for its own slice."""
                    xr = stage.tile([P, KD * CH], BF16, tag="xr", bufs=3,
                                    name="xr")
                    for j in range(KD):
                        nc.vector.tensor_copy(
                            out=xr[:, j * CH:j * CH + w],
                            in_=xr32[:, j * CH:j * CH + w])
                    return xr

                # K^T first (phase 2's first score groups need it earliest)
                for c in range(SH // CH):
                    xr = cast_chunk(xrk32[c], CH)
                    for jo in range(KD):
                        pq = ppsum.tile([P, CH], F32, tag="pq", name="pqk")
                        for kd in range(KD):
                            nc.tensor.matmul(
                                pq,
                                wk[:, kd * D + jo * P: kd * D + (jo + 1) * P],
                                xr[:, kd * CH:(kd + 1) * CH],
                                start=(kd == 0), stop=(kd == KD - 1))
                        nc.scalar.activation(
                            out=kT[:, jo, c * CH:(c + 1) * CH],
                            in_=pq, func=COPY)
                    if c == 0:
                        for j in range(KD):
                            nc.vector.tensor_copy(
                                out=wq[:, j * D:(j + 1) * D],
                                in_=wq32[:, j * D:(j + 1) * D])
                        nc.vector.tensor_copy(out=wv, in_=wv32)

                # Q^T and V from the compacted x^T, chunk by chunk
                for ci, (c0, w) in enumerate(qchunks):
                    xr32 = stage.tile([P, KD * CH], F32R, tag="xr32", bufs=4,
                                      name="xrq32")
                    for j in range(KD):
                        dmae[(ci + j) % 2].dma_start(
                            out=xr32[:, j * CH:j * CH + w],
                            in_=xkT[j * P:(j + 1) * P, c0:c0 + w])
                    xr = cast_chunk(xr32, w)
                    for jo in range(KD):
                        pq = ppsum.tile([P, CH], F32, tag="pq")
                        for kd in range(KD):
                            nc.tensor.matmul(
                                pq[:, 0:w],
                                wq[:, kd * D + jo * P: kd * D + (jo + 1) * P],
                                xr[:, kd * CH:kd * CH + w],
                                start=(kd == 0), stop=(kd == KD - 1))
                        nc.scalar.activation(
                            out=qT[:, jo, c0:c0 + w],
                            in_=pq[:, 0:w], func=COPY)
                    for tt in range(w // P):
                        ti = c0 // P + tt
                        pv = ppsum.tile([P, D], F32, tag="pv")
                        for kd in range(KD):
                            nc.tensor.matmul(
                                pv,
                                xr[:, kd * CH + tt * P: kd * CH + (tt + 1) * P],
                                wv[:, kd * D:(kd + 1) * D],
                                start=(kd == 0), stop=(kd == KD - 1))
                        nc.scalar.activation(
                            out=vA[:, ti, :], in_=pv, func=COPY)

            # ---------------- phase 2: attention ----------------
            with tc.tile_pool(name="att", bufs=1) as att, \
                 tc.tile_pool(name="apsum", bufs=1, space="PSUM") as apsum:

                nchunk = SH // CH
                pending = [None]     # delayed epilogue from previous chunk

                def make_epilogue(den128, osb, sc):
                    def emit():
                        # den[s] = column sum of den128 (P^T already masked
                        # by the EXP bias), then out *= 1/den via a rank-1
                        # broadcast matmul; the scale mult reads the
                        # broadcast PSUM directly (no SBUF staging copy)
                        dps = apsum.tile([1, CH], F32, tag="bc", name="dps")
                        nc.tensor.matmul(dps, onec, den128,
                                         start=True, stop=True)
                        rec = att.tile([1, CH], F32, tag="rec")
                        nc.vector.reciprocal_approx_fast(out=rec, in_=dps)
                        recr = att.tile([1, CH], F32R, tag="recr")
                        nc.vector.tensor_copy(out=recr, in_=rec)
                        bps = apsum.tile([P, CH], F32, tag="bc", name="bps")
                        nc.tensor.matmul(bps, ones, recr,
                                         start=True, stop=True)
                        dma_engs = [nc.sync, nc.gpsimd, nc.scalar, nc.sync]
                        for d in range(KD):
                            fin = att.tile([P, CH], F32, tag=f"fin{d % 2}",
                                           name=f"fin{d}", bufs=2)
                            nc.vector.tensor_mul(fin, osb[d], bps)
                            dma_engs[d].dma_start(
                                out=outT[d * P:(d + 1) * P,
                                         sc * CH:(sc + 1) * CH],
                                in_=fin)
                    return emit

                for sc in range(nchunk):
                    opsum = [apsum.tile([P, CH], F32, tag=f"o{d}",
                                        name=f"opsum{d}")
                             for d in range(KD)]
                    # den128 accumulates P^T on the DVE (off the PE); bufs=2
                    # because the delayed den matmul still reads chunk sc's
                    # accumulator while chunk sc+1 starts a fresh one
                    den128 = att.tile([P, CH], F32R, tag="den128", bufs=2)

                    def s_group(ti, sc=sc):
                        ss = apsum.tile([P, CH], F32, tag="s", bufs=3)
                        for kd in range(KD):
                            nc.tensor.matmul(
                                ss,
                                qT[:, kd, ti * P:(ti + 1) * P],
                                kT[:, kd, sc * CH:(sc + 1) * CH],
                                start=(kd == 0), stop=(kd == KD - 1))
                        return ss

                    ss_cur = s_group(0)
                    for ti in range(ntk):
                        ss_next = s_group(ti + 1) if ti + 1 < ntk else None
                        pt = att.tile([P, CH], BF16, tag="pt", bufs=3)
                        # masked softmax numerator: exp(score*scale + mbias)
                        nc.scalar.activation(out=pt, in_=ss_cur, func=EXP,
                                             scale=SCALE,
                                             bias=mbias[:, ti:ti + 1])
                        for d in range(KD):
                            nc.tensor.matmul(
                                opsum[d],
                                vA[:, ti, d * P:(d + 1) * P],
                                pt, start=(ti == 0), stop=(ti == ntk - 1))
                        if ti == 0:
                            nc.vector.tensor_copy(out=den128, in_=pt)
                        else:
                            nc.vector.tensor_add(den128, den128, pt)
                        if ti == 2 and pending[0] is not None:
                            # previous chunk's epilogue: its den/broadcast
                            # matmuls slot in here so the PE never idles
                            # waiting on the DVE reciprocal chain
                            pending[0]()
                            pending[0] = None
                        ss_cur = ss_next

                    if sc < nchunk - 1:
                        # drain the AV accumulators now (frees the PSUM
                        # banks for the next chunk); the rest of the
                        # epilogue waits for the next chunk's stream
                        osb = []
                        for d in range(KD):
                            ot = att.tile([P, CH], F32, tag=f"osb{d}",
                                          name=f"osb{d}")
                            nc.vector.tensor_copy(out=ot, in_=opsum[d])
                            osb.append(ot)
                        pending[0] = make_epilogue(den128, osb, sc)
                    else:
                        # last chunk: run the reciprocal chain FIRST (the
                        # drains would otherwise queue ahead of it on the
                        # DVE FIFO and stretch the exposed tail), then
                        # interleave drain(d) -> scale(d) -> store(d) so
                        # each output block ships as soon as it's ready
                        dps = apsum.tile([1, CH], F32, tag="bc", name="dps")
                        nc.tensor.matmul(dps, onec, den128,
                                         start=True, stop=True)
                        rec = att.tile([1, CH], F32, tag="rec")
                        nc.vector.reciprocal_approx_fast(out=rec, in_=dps)
                        recr = att.tile([1, CH], F32R, tag="recr")
                        nc.vector.tensor_copy(out=recr, in_=rec)
                        bps = apsum.tile([P, CH], F32, tag="bc", name="bps")
                        nc.tensor.matmul(bps, ones, recr,
                                         start=True, stop=True)
                        dma_engs = [nc.sync, nc.gpsimd, nc.scalar, nc.sync]
                        for d in range(KD):
                            ot = att.tile([P, CH], F32, tag=f"osb{d}",
                                          name=f"osb{d}")
                            nc.vector.tensor_copy(out=ot, in_=opsum[d])
                            fin = att.tile([P, CH], F32, tag=f"fin{d % 2}",
                                           name=f"fin{d}", bufs=2)
                            nc.vector.tensor_mul(fin, ot, bps)
                            dma_engs[d].dma_start(
                                out=outT[d * P:(d + 1) * P,
                                         sc * CH:(sc + 1) * CH],
                                in_=fin)

    nc.compile()
    return nc


def _prep(x, mask, Wk, Wq, Wv):
    """Host-side layout prep: transposes + mask-compaction gather.
    Returns (skp, in_maps)."""
    x = np.asarray(x, dtype=np.float32)
    mask_np = np.asarray(mask)
    wqT = np.ascontiguousarray(np.asarray(Wq, dtype=np.float32).T)
    wkT = np.ascontiguousarray(np.asarray(Wk, dtype=np.float32).T)
    wvT = np.ascontiguousarray(np.asarray(Wv, dtype=np.float32).T)

    idxs = [np.nonzero(mask_np[b])[0] for b in range(B)]
    nk_max = max(len(ix) for ix in idxs)
    skp = max(256, ((nk_max + 127) // 128) * 128)
    ntk = skp // P

    in_maps = []
    for b in range(B):
        xT = np.ascontiguousarray(x[b].T)                  # [D, S]
        xk = np.zeros((D, skp), dtype=np.float32)
        xk[:, :len(idxs[b])] = xT[:, idxs[b]]
        mg = np.zeros(skp, dtype=np.float32)
        mg[:len(idxs[b])] = 1.0
        mkT = np.ascontiguousarray(mg.reshape(ntk, P).T)   # [P, ntk]
        for h in range(2):
            in_maps.append({
                "xsT": np.ascontiguousarray(xT[:, h * SH:(h + 1) * SH]),
                "xkT": xk,
                "wqT": wqT, "wkT": wkT, "wvT": wvT,
                "maskT": mkT,
            })
    return skp, in_maps


def _get_nc(skp):
    if skp not in _CACHE:
        _CACHE[skp] = _build(skp)
    return _CACHE[skp]


def kernel(x, mask, Wk, Wq, Wv):
    skp, in_maps = _prep(x, mask, Wk, Wq, Wv)
    nc = _get_nc(skp)

    res = run_bass_kernel_spmd(nc, in_maps, core_ids=list(range(8)))

    out = np.empty((B, S, D), dtype=np.float32)
    for b in range(B):
        for h in range(2):
            out[b, h * SH:(h + 1) * SH, :] = res.results[2 * b + h]["outT"].T
    return out
